# revision 48
# baseline (speedup 1.0000x reference)
"""GAT layer (edge softmax + weighted scatter) on 8 Trainium2 NeuronCores, v3.

Strategy (dst-range sharding, no collectives):
  - Nodes split into 8 contiguous dst ranges of 6250; dst is sorted, so each
    core owns a contiguous edge range and all of its destination segments.
  - Fixed 32-node window grid (196 windows/core). Edges of each window are
    split by src < 32768 (int16 gather limit) and chunked into <=128-edge
    chunks. Chunk counts per (window, stream) are maxed across cores so all
    8 cores share one compiled schedule; chunks run K=32 per super-step
    (lo-stream supersteps first, then hi).
  - Gather table is bf16 [N, 128]: row n = [h[n] (64) | 1.0 | zeros(63)].
    Col 64 provides the softmax-denominator ones column for free; bf16 makes
    the scatter matmuls single-pass (fp32 double-pumps the PE array).
  - Per super-step: 4x 1024-idx dma_gather pulls bf16 rows; scores
    e = rowsum(Z*w1) on DVE (bf16 mult + reduce); E = e + a_dst[window cols]
    (a_dst tile broadcast from a preamble-built table); leaky_relu and exp
    run on the Scalar engine (Lrelu + Exp); Sp = P * onehot-mask where the
    mask is HOST-built metadata DMA'd per super-step (no is_equal on DVE).
  - Scatter: matmul lhsT=Sp[:,c,:] [128,32] bf16, rhs=Z[:,c,0:65] bf16 into
    a PSUM-RESIDENT accumulator: 196 windows live across 7 PSUM banks
    (4 partition-groups x 7 col-groups of [32,65] each); start only on a
    window's first chunk, stop on its last. No per-run drains.
  - Epilogue: 7 whole-bank Scalar-engine drains to SBUF, divide features by
    the denominator column, one DMA writes the (window-permuted) output;
    the host inverse-permutes rows.
"""
import sys

sys.path.insert(0, "/opt/trn_rl_repo")

import numpy as np
import ml_dtypes

BF16 = ml_dtypes.bfloat16

N, F, E, NCORES = 50000, 64, 800000, 8
NLOC = N // NCORES            # 6250 nodes per core
K = 32                        # chunks per super-step
W = 32                        # window size (dst nodes per chunk)
NPAD = 6272                   # 128 * 49
NWIN = NPAD // W              # 196
HALF = 32768                  # int16 split of the gather table
NEG_SLOPE = 0.01
DUMP = NWIN                   # dump window id (pad chunks)
NBANK = 7                     # PSUM banks holding windows (196 = 7*28)


# ---------------------------------------------------------------- host prep
def _wrap16(flat):
    """dma/ap_gather idx layout: idx k at (partition k%16, col k//16),
    replicated across the 8 q7 cores (partition groups of 16)."""
    a = np.asarray(flat, np.int16).reshape(-1, 16).T
    return np.ascontiguousarray(np.tile(a, (8, 1)), dtype=np.int16)


def _prep(src, dst):
    """Split per core / window / stream; find shared per-window chunk counts."""
    cores = []
    for c in range(NCORES):
        n0 = c * NLOC
        e0, e1 = np.searchsorted(dst, [n0, n0 + NLOC])
        s_loc = src[e0:e1].astype(np.int64)
        d_loc = (dst[e0:e1] - n0).astype(np.int64)
        counts = np.bincount(d_loc // W, minlength=NWIN)
        ends = np.cumsum(counts)
        starts = ends - counts
        per_win = []
        for w in range(NWIN):
            sl = slice(starts[w], ends[w])
            s_w, d_w = s_loc[sl], d_loc[sl] - W * w
            m = s_w < HALF
            per_win.append(((s_w[m], d_w[m]), (s_w[~m] - HALF, d_w[~m])))
        cores.append(per_win)

    nch = np.zeros((NWIN, 2), np.int64)
    for per_win in cores:
        for w in range(NWIN):
            for st in (0, 1):
                nch[w, st] = max(nch[w, st],
                                 -(-len(per_win[w][st][0]) // 128))
    nch[nch.sum(1) == 0, 0] = 1      # >=1 chunk per window (PSUM init)
    return cores, nch


def _schedule(nch):
    """seq[pos] = (window, chunk_i, stream, start, stop) shared by all cores.

    Each dma_gather covers 8 chunks and picks its own table half, so the
    lo/hi stream split is packed at QUEUE-GROUP granularity (multiples of 8
    chunks, not of K=32): 988 real chunks -> 124 groups -> 31 supersteps.

    PSUM start_tensor_calc zeroes the ENTIRE 2KB bank row (the "zero
    region") on the written partitions, so windows sharing a (bank,
    partition-group) row must form ONE accumulation group: start fires only
    on the row-group's very first chunk, stop on its last."""
    groups = []
    for st in (0, 1):
        lst = []
        for w in range(NWIN):
            for i in range(nch[w, st]):
                lst.append((w, i, st))
        ng = -(-len(lst) // 8)
        lst += [(DUMP, 0, st)] * (ng * 8 - len(lst))
        for g in range(ng):
            groups.append(lst[8 * g : 8 * g + 8])
    n_sup = -(-len(groups) // 4)
    groups += [[(DUMP, 0, 0)] * 8] * (n_sup * 4 - len(groups))
    qstream = [[groups[4 * s + q][0][2] for q in range(4)]
               for s in range(n_sup)]
    flat = [ent for grp in groups for ent in grp]
    s_lo, s_hi = n_sup, 0
    # row-group of window w: (bank w%7, partition-group (w//7)%4)
    first_pos, last_pos = {}, {}
    for pos, (w, i, st) in enumerate(flat):
        if w == DUMP:
            continue
        rg = (w % NBANK, (w // NBANK) % 4)
        if rg not in first_pos:
            first_pos[rg] = pos
        last_pos[rg] = pos
    seq = []
    for pos, (w, i, st) in enumerate(flat):
        if w == DUMP:
            seq.append((w, i, st, True, True))
        else:
            rg = (w % NBANK, (w // NBANK) % 4)
            seq.append((w, i, st, first_pos[rg] == pos, last_pos[rg] == pos))
    return seq, qstream, s_lo, s_hi


def _build_arrays(per_win, seq, qstream, s_lo, s_hi):
    """Per-core packed [S,128,256+K*W] bf16: idx i16 bits | onehot mask."""
    S = s_lo + s_hi
    # pad slots must gather SOME valid row (mask=0 nullifies them); spread
    # them across the table — row-0 defaults serialize on one HBM bank
    idxg = np.empty((S, 4096), np.int64)
    for s in range(S):
        for q in range(4):
            lim = HALF if qstream[s][q] == 0 else N - HALF
            idxg[s, 1024 * q : 1024 * (q + 1)] = (
                np.arange(1024, dtype=np.int64) * 401 + s * 127 + q * 31) % lim
    mask = np.zeros((S, 128, K * W), BF16)
    skp = -(-S * K // 128) * 128
    aidx = np.full((skp,), NWIN, np.int64)
    for pos, (w, i, st, _f, _l) in enumerate(seq):
        s, c = pos // K, pos % K
        if w == DUMP:
            continue
        ss, dd = per_win[w][st]
        ss, dd = ss[128 * i : 128 * i + 128], dd[128 * i : 128 * i + 128]
        ec = len(ss)
        idxg[s, c * 128 : c * 128 + ec] = ss
        mask[s, np.arange(ec), c * W + dd] = 1
        aidx[s * K + c] = w
    packed = np.empty((S, 128, 256 + K * W), np.uint16)
    for s in range(S):
        packed[s, :, 0:256] = _wrap16(idxg[s]).view(np.uint16)
    packed[:, :, 256:] = mask.view(np.uint16)
    return packed.view(BF16), _wrap16(aidx).view(np.float32)


# ------------------------------------------------------------- bass program
def _build_program(s_lo, s_hi, seq, qstream):
    import concourse.bacc as bacc
    import concourse.tile as tile
    import concourse.mybir as mybir
    from concourse import bass

    f32, i16, bf16 = mybir.dt.float32, mybir.dt.int16, mybir.dt.bfloat16
    AF = mybir.ActivationFunctionType
    OP = mybir.AluOpType
    S = s_lo + s_hi

    nc = bacc.Bacc("TRN2", target_bir_lowering=False, debug=False,
                   num_devices=NCORES, num_swdge_queues=4)
    hb_t = nc.dram_tensor("hb", [N, 128], bf16, kind="ExternalInput")
    hs_t = nc.dram_tensor("h_slice", [NPAD, F], f32, kind="ExternalInput")
    w_t = nc.dram_tensor("attn_w", [2 * F], f32, kind="ExternalInput")
    # cols 0:256 = gather idx (i16 bits in a bf16 container), 256:1280 = mask
    pk_t = nc.dram_tensor("packed", [S, 128, 256 + K * W], bf16,
                          kind="ExternalInput")
    SKP = -(-S * K // 128) * 128
    aw_t = nc.dram_tensor("aw", [128, SKP // 32], f32, kind="ExternalInput")
    out_t = nc.dram_tensor("out", [NPAD, F], f32, kind="ExternalOutput")
    adr_t = nc.dram_tensor("adr", [NPAD], f32, kind="Internal")
    ta_t = nc.dram_tensor("ta", [NWIN + 1, F], f32, kind="Internal")
    a2_t = nc.dram_tensor("a2", [SKP, W], bf16, kind="Internal")

    def bc_ap(tensor, offset, ap):
        return bass.AP(tensor=tensor, offset=offset, ap=ap)

    with tile.TileContext(nc) as tc:
        with tc.tile_pool(name="const", bufs=1) as const, \
             tc.tile_pool(name="pre", bufs=1) as pre, \
             tc.tile_pool(name="ps", bufs=1, space="PSUM") as ps:

            # ---------------- constants
            w1f = const.tile([128, F], f32)
            nc.gpsimd.dma_start(out=w1f[:], in_=bc_ap(w_t, 0, [[0, 128], [1, F]]))
            w2t = const.tile([128, F], f32)
            nc.gpsimd.dma_start(out=w2t[:], in_=bc_ap(w_t, F, [[0, 128], [1, F]]))
            w1b = const.tile([128, F], bf16)
            nc.vector.tensor_copy(w1b[:], w1f[:])
            # physically replicated w1 over the chunk dim: keeps the zw
            # multiply's inputs step-1 contiguous so the DVE picks 2x mode
            # (a stride-0 broadcast AP drops it to 1x)
            w1r = const.tile([128, K, F], bf16)
            nc.vector.tensor_copy(
                w1r[:], w1b[:, None, :].to_broadcast([128, K, F]))

            # ---------------- preamble: a_dst table -> per-chunk A rows (bf16)
            with tc.tile_pool(name="pre2", bufs=1) as pre2:
                hs = pre2.tile([128, NPAD // 128, F], f32)
                nc.sync.dma_start(
                    out=hs[:], in_=hs_t[:].rearrange("(p t) f -> p t f", p=128))
                nc.vector.tensor_tensor(
                    out=hs[:], in0=hs[:],
                    in1=w2t[:, None, :].to_broadcast([128, NPAD // 128, F]),
                    op=OP.mult)
                a_sb = pre2.tile([128, NPAD // 128], f32)
                nc.vector.tensor_reduce(out=a_sb[:], in_=hs[:],
                                        axis=mybir.AxisListType.X, op=OP.add)
                nc.sync.dma_start(
                    out=adr_t[:].rearrange("(p t) -> p t", p=128), in_=a_sb[:])
                a_row = pre2.tile([1, NPAD], f32)
                nc.sync.dma_start(out=a_row[:],
                                  in_=bc_ap(adr_t, 0, [[0, 1], [1, NPAD]]))
                # ta_t row w = a_dst[32w .. 32w+32] (cols 32:64 unused);
                # row NWIN = zeros (dump chunks)
                nc.sync.dma_start(
                    out=ta_t[0:NWIN, 0:W],
                    in_=a_row[0:1, :].rearrange("p (w j) -> p w j", j=W))
                zrow = pre2.tile([128, W], f32)
                nc.vector.memset(zrow[:], 0.0)
                nc.sync.dma_start(out=ta_t[NWIN : NWIN + 1, 0:W],
                                  in_=zrow[0:1, :])
                # cols W:2W are gathered (256B elems) but unused — keep them
                # initialized so CoreSim's finiteness checks pass
                nc.sync.dma_start(
                    out=ta_t[:, W : 2 * W],
                    in_=zrow[0:1, None, :].to_broadcast([1, NWIN + 1, W]))
                awi = pre2.tile([128, SKP // 32], f32)
                nc.sync.dma_start(out=awi[:], in_=aw_t[:])
                At = pre2.tile([128, SKP // 128, F], f32)
                nc.gpsimd.dma_gather(
                    out_ap=At[:], in_ap=ta_t[:],
                    idxs_ap=awi[:].bitcast(i16), num_idxs=SKP,
                    num_idxs_reg=SKP, elem_size=F, queue_num=0)
                A2 = pre2.tile([128, SKP // 128, W], bf16)
                nc.vector.tensor_copy(A2[:], At[:, :, 0:W])
                nc.sync.dma_start(
                    out=a2_t[:].rearrange("(c p) w -> p c w", p=128),
                    in_=A2[:])

            # ---------------- resident PSUM window accumulators
            # window w -> bank w%7, slot w//7: partition group (w//7)%4,
            # col group (w//7)//4. bank 7 = dump target for pad chunks.
            banks = [ps.tile([128, 512], f32, name=f"bank{b}", tag=f"bank{b}")
                     for b in range(8)]

            def bank_region(w):
                if w == DUMP:
                    return banks[7][0:32, 0:65], (0, 0)
                slot = w // NBANK
                p0, c0 = 32 * (slot % 4), 65 * (slot // 4)
                return banks[w % NBANK][p0 : p0 + 32, c0 : c0 + 65], (0, p0)

            # ---------------- super-steps (2-stage software pipeline)
            # stage A(s): gathers + DVE score prep + scalar Lrelu/Exp
            # stage B(s): Sp = P*mask, scatter matmuls — emitted one
            # iteration later so the scalar round trip never stalls DVE.
            from contextlib import ExitStack
            lctx = ExitStack()
            ldi = lctx.enter_context(tc.tile_pool(name="ldi", bufs=5))
            zp = lctx.enter_context(tc.tile_pool(name="zp", bufs=5))
            b3 = lctx.enter_context(tc.tile_pool(name="b3", bufs=4))
            med = lctx.enter_context(tc.tile_pool(name="med", bufs=4))

            stash = {}
            for it in range(S + 1):
                if it < S:
                    s = it
                    ld = ldi.tile([128, 256 + K * W], bf16, tag="ld")
                    nc.sync.dma_start(out=ld[:], in_=pk_t[s])
                    ig = ld[:, 0:256].bitcast(i16)

                    Z = zp.tile([128, K, 128], bf16, tag="Z")
                    for q in range(4):
                        tab = (hb_t[0:HALF, :] if qstream[s][q] == 0
                               else hb_t[HALF:N, :])
                        nc.gpsimd.dma_gather(
                            out_ap=Z[:, 8 * q : 8 * q + 8, :],
                            in_ap=tab,
                            idxs_ap=ig[:, 64 * q : 64 * q + 64],
                            num_idxs=1024, num_idxs_reg=1024, elem_size=128,
                            queue_num=q)

                    Mt = ld[:, 256:].rearrange("p (c w) -> p c w", w=W)
                    # A[c, w] = a_dst[32*w_c + w], partition-replicated
                    A = med.tile([128, K, W], bf16, tag="A")
                    nc.sync.dma_start(
                        out=A[:],
                        in_=bc_ap(a2_t, s * K * W,
                                  [[0, 128], [W, K], [1, W]]))

                    # e = rowsum(Z * w1) — DVE reduce accumulates fp32
                    # internally; bf16 output rounding is within tolerance
                    zw = med.tile([128, K, F], bf16, tag="zw")
                    nc.vector.tensor_tensor(
                        out=zw[:], in0=Z[:, :, 0:F], in1=w1r[:],
                        op=OP.mult)
                    sCb = med.tile([128, K], bf16, tag="sCb")
                    with nc.allow_low_precision("bf16 scores within 2e-2 gate"):
                        nc.vector.tensor_reduce(out=sCb[:], in_=zw[:],
                                                axis=mybir.AxisListType.X,
                                                op=OP.add)

                    # E = e + a_dst; leaky = max(E, 0.01E) (ACT Copy shares
                    # Exp's table set — Lrelu does not and thrashes loads)
                    Emat = b3.tile([128, K, W], bf16, tag="Emat")
                    nc.vector.tensor_tensor(
                        out=Emat[:],
                        in0=sCb[:, :, None].to_broadcast([128, K, W]),
                        in1=A[:], op=OP.add)
                    El = b3.tile([128, K, W], bf16, tag="El")
                    nc.scalar.activation(out=El[:], in_=Emat[:], func=AF.Copy,
                                         scale=NEG_SLOPE)
                    nc.vector.tensor_tensor(out=El[:], in0=El[:], in1=Emat[:],
                                            op=OP.max)
                    Pm = b3.tile([128, K, W], bf16, tag="Pm")
                    nc.scalar.activation(out=Pm[:], in_=El[:], func=AF.Exp)
                    stash[s] = (Z, Pm, Mt)

                if it >= 1:
                    s = it - 1
                    Z, Pm, Mtv = stash.pop(s)
                    Sp = b3.tile([128, K, W], bf16, tag="Sp")
                    nc.vector.tensor_tensor(out=Sp[:], in0=Pm[:], in1=Mtv,
                                            op=OP.mult)
                    for c in range(K):
                        w, _i, _st, first, last = seq[s * K + c]
                        reg, tpos = bank_region(w)
                        nc.tensor.matmul(out=reg, lhsT=Sp[:, c, :],
                                         rhs=Z[:, c, 0:F + 1],
                                         start=first, stop=last,
                                         tile_position=tpos)

            # ---------------- epilogue: drain banks, divide by denominator
            acc = pre.tile([128, NBANK, 28 // 4 * 65], f32)
            for b in range(NBANK):
                nc.scalar.copy(out=acc[:, b, :], in_=banks[b][:, 0 : 455])
            accv = acc[:].rearrange("p b (k x) -> p b k x", x=65)
            rmax = pre.tile([128, NBANK, 7], f32)
            nc.vector.tensor_scalar_max(rmax[:], accv[:, :, :, F], 1e-30)
            rcp = pre.tile([128, NBANK, 7], f32)
            nc.vector.reciprocal(rcp[:], rmax[:])
            nc.vector.tensor_tensor(
                out=accv[:, :, :, 0:F], in0=accv[:, :, :, 0:F],
                in1=rcp[:, :, :, None].to_broadcast([128, NBANK, 7, F]),
                op=OP.mult)
            # out rows in (b, k, g, r) device order; host inverse-permutes
            nc.sync.dma_start(
                out=out_t[:].rearrange("(b k g r) f -> (g r) b k f",
                                       b=NBANK, k=7, g=4),
                in_=accv[:, :, :, 0:F])
            lctx.close()
    nc.compile()
    return nc


_prog_cache = {}
_last_in_maps = None
_last_res = None


def kernel(h, attn_w, src, dst):
    from concourse.bass_utils import run_bass_kernel_spmd

    h = np.ascontiguousarray(h, dtype=np.float32)
    attn_w = np.ascontiguousarray(attn_w, dtype=np.float32)
    src = np.asarray(src, dtype=np.int32)
    dst = np.asarray(dst, dtype=np.int32)

    cores, nch = _prep(src, dst)
    seq, qstream, s_lo, s_hi = _schedule(nch)

    key = (s_lo, s_hi, tuple(seq), tuple(map(tuple, qstream)))
    if key not in _prog_cache:
        _prog_cache[key] = _build_program(s_lo, s_hi, seq, qstream)
    nc = _prog_cache[key]

    # bf16 gather table: row n = [h[n] | 1.0 | zeros]; col 64 is the
    # softmax-denominator ones column
    hb = np.zeros((N, 128), BF16)
    hb[:, :F] = h
    hb[:, F] = 1.0

    in_maps = []
    for d in range(NCORES):
        n0 = d * NLOC
        packed, aw = _build_arrays(cores[d], seq, qstream, s_lo, s_hi)
        h_slice = np.zeros((NPAD, F), np.float32)
        h_slice[:NLOC] = h[n0 : n0 + NLOC]
        in_maps.append({
            "hb": hb,
            "h_slice": h_slice,
            "attn_w": attn_w,
            "packed": packed,
            "aw": aw,
        })

    global _last_in_maps, _last_res
    _last_in_maps = in_maps
    res = run_bass_kernel_spmd(nc, in_maps, list(range(NCORES)))
    _last_res = res
    # device rows are (bank, colgroup, partgroup, row): window w = 7*slot+b
    # with slot = 4*k+g lives at device row ((b*7+k)*4+g)*32+r
    b, k, g, r = np.meshgrid(np.arange(NBANK), np.arange(7), np.arange(4),
                             np.arange(32), indexing="ij")
    node = 32 * (NBANK * (4 * k + g) + b) + r
    inv = np.empty(NPAD, np.int64)
    inv[node.ravel()] = np.arange(NPAD)
    out = np.concatenate(
        [res.results[d]["out"][inv[:NLOC]] for d in range(NCORES)])
    return out.astype(np.float32)


if __name__ == "__main__":
    import reference

    inputs = reference.setup_inputs()
    inputs = {k: np.asarray(v) for k, v in inputs.items()}
    got = kernel(**inputs)
    exp = np.asarray(reference.reference(**inputs))
    denom = np.abs(exp).max()
    rel = np.abs(got - exp).max() / denom
    print("Relative error:", rel)


# revision 49
# speedup vs baseline: 1.0079x; 1.0079x over previous
"""GAT layer (edge softmax + weighted scatter) on 8 Trainium2 NeuronCores, v3.

Strategy (dst-range sharding, no collectives):
  - Nodes split into 8 contiguous dst ranges of 6250; dst is sorted, so each
    core owns a contiguous edge range and all of its destination segments.
  - Fixed 32-node window grid (196 windows/core). Edges of each window are
    split by src < 32768 (int16 gather limit) and chunked into <=128-edge
    chunks. Chunk counts per (window, stream) are maxed across cores so all
    8 cores share one compiled schedule; chunks run K=32 per super-step
    (lo-stream supersteps first, then hi).
  - Gather table is bf16 [N, 128]: row n = [h[n] (64) | 1.0 | zeros(63)].
    Col 64 provides the softmax-denominator ones column for free; bf16 makes
    the scatter matmuls single-pass (fp32 double-pumps the PE array).
  - Per super-step: 4x 1024-idx dma_gather pulls bf16 rows; scores
    e = rowsum(Z*w1) on DVE (bf16 mult + reduce); E = e + a_dst[window cols]
    (a_dst tile broadcast from a preamble-built table); leaky_relu and exp
    run on the Scalar engine (Lrelu + Exp); Sp = P * onehot-mask where the
    mask is HOST-built metadata DMA'd per super-step (no is_equal on DVE).
  - Scatter: matmul lhsT=Sp[:,c,:] [128,32] bf16, rhs=Z[:,c,0:65] bf16 into
    a PSUM-RESIDENT accumulator: 196 windows live across 7 PSUM banks
    (4 partition-groups x 7 col-groups of [32,65] each); start only on a
    window's first chunk, stop on its last. No per-run drains.
  - Epilogue: 7 whole-bank Scalar-engine drains to SBUF, divide features by
    the denominator column, one DMA writes the (window-permuted) output;
    the host inverse-permutes rows.
"""
import sys

sys.path.insert(0, "/opt/trn_rl_repo")

import numpy as np
import ml_dtypes

BF16 = ml_dtypes.bfloat16

N, F, E, NCORES = 50000, 64, 800000, 8
NLOC = N // NCORES            # 6250 nodes per core
K = 32                        # chunks per super-step
W = 32                        # window size (dst nodes per chunk)
NPAD = 6272                   # 128 * 49
NWIN = NPAD // W              # 196
HALF = 32768                  # int16 split of the gather table
NEG_SLOPE = 0.01
DUMP = NWIN                   # dump window id (pad chunks)
NBANK = 7                     # PSUM banks holding windows (196 = 7*28)


# ---------------------------------------------------------------- host prep
def _wrap16(flat):
    """dma/ap_gather idx layout: idx k at (partition k%16, col k//16),
    replicated across the 8 q7 cores (partition groups of 16)."""
    a = np.asarray(flat, np.int16).reshape(-1, 16).T
    return np.ascontiguousarray(np.tile(a, (8, 1)), dtype=np.int16)


def _prep(src, dst):
    """Split per core / window / stream; find shared per-window chunk counts."""
    cores = []
    for c in range(NCORES):
        n0 = c * NLOC
        e0, e1 = np.searchsorted(dst, [n0, n0 + NLOC])
        s_loc = src[e0:e1].astype(np.int64)
        d_loc = (dst[e0:e1] - n0).astype(np.int64)
        counts = np.bincount(d_loc // W, minlength=NWIN)
        ends = np.cumsum(counts)
        starts = ends - counts
        per_win = []
        for w in range(NWIN):
            sl = slice(starts[w], ends[w])
            s_w, d_w = s_loc[sl], d_loc[sl] - W * w
            m = s_w < HALF
            per_win.append(((s_w[m], d_w[m]), (s_w[~m] - HALF, d_w[~m])))
        cores.append(per_win)

    nch = np.zeros((NWIN, 2), np.int64)
    for per_win in cores:
        for w in range(NWIN):
            for st in (0, 1):
                nch[w, st] = max(nch[w, st],
                                 -(-len(per_win[w][st][0]) // 128))
    nch[nch.sum(1) == 0, 0] = 1      # >=1 chunk per window (PSUM init)
    return cores, nch


def _schedule(nch):
    """seq[pos] = (window, chunk_i, stream, start, stop) shared by all cores.

    Each dma_gather covers 8 chunks and picks its own table half, so the
    lo/hi stream split is packed at QUEUE-GROUP granularity (multiples of 8
    chunks, not of K=32): 988 real chunks -> 124 groups -> 31 supersteps.

    PSUM start_tensor_calc zeroes the ENTIRE 2KB bank row (the "zero
    region") on the written partitions, so windows sharing a (bank,
    partition-group) row must form ONE accumulation group: start fires only
    on the row-group's very first chunk, stop on its last."""
    groups = []
    for st in (0, 1):
        lst = []
        for w in range(NWIN):
            for i in range(nch[w, st]):
                lst.append((w, i, st))
        ng = -(-len(lst) // 8)
        lst += [(DUMP, 0, st)] * (ng * 8 - len(lst))
        for g in range(ng):
            groups.append(lst[8 * g : 8 * g + 8])
    n_sup = -(-len(groups) // 4)
    groups += [[(DUMP, 0, 0)] * 8] * (n_sup * 4 - len(groups))
    qstream = [[groups[4 * s + q][0][2] for q in range(4)]
               for s in range(n_sup)]
    flat = [ent for grp in groups for ent in grp]
    s_lo, s_hi = n_sup, 0
    # row-group of window w: (bank w%7, partition-group (w//7)%4)
    first_pos, last_pos = {}, {}
    for pos, (w, i, st) in enumerate(flat):
        if w == DUMP:
            continue
        rg = (w % NBANK, (w // NBANK) % 4)
        if rg not in first_pos:
            first_pos[rg] = pos
        last_pos[rg] = pos
    seq = []
    for pos, (w, i, st) in enumerate(flat):
        if w == DUMP:
            seq.append((w, i, st, True, True))
        else:
            rg = (w % NBANK, (w // NBANK) % 4)
            seq.append((w, i, st, first_pos[rg] == pos, last_pos[rg] == pos))
    return seq, qstream, s_lo, s_hi


def _build_arrays(per_win, seq, qstream, s_lo, s_hi):
    """Per-core packed [S,128,256+K*W] bf16: idx i16 bits | onehot mask."""
    S = s_lo + s_hi
    # pad slots must gather SOME valid row (mask=0 nullifies them); spread
    # them across the table — row-0 defaults serialize on one HBM bank
    idxg = np.empty((S, 4096), np.int64)
    for s in range(S):
        for q in range(4):
            lim = HALF if qstream[s][q] == 0 else N - HALF
            idxg[s, 1024 * q : 1024 * (q + 1)] = (
                np.arange(1024, dtype=np.int64) * 401 + s * 127 + q * 31) % lim
    mask = np.zeros((S, 128, K * W), BF16)
    skp = -(-S * K // 128) * 128
    aidx = np.full((skp,), NWIN, np.int64)
    for pos, (w, i, st, _f, _l) in enumerate(seq):
        s, c = pos // K, pos % K
        if w == DUMP:
            continue
        ss, dd = per_win[w][st]
        ss, dd = ss[128 * i : 128 * i + 128], dd[128 * i : 128 * i + 128]
        ec = len(ss)
        idxg[s, c * 128 : c * 128 + ec] = ss
        mask[s, np.arange(ec), c * W + dd] = 1
        aidx[s * K + c] = w
    packed = np.empty((S, 128, 256 + K * W), np.uint16)
    for s in range(S):
        packed[s, :, 0:256] = _wrap16(idxg[s]).view(np.uint16)
    packed[:, :, 256:] = mask.view(np.uint16)
    return packed.view(BF16), _wrap16(aidx).view(np.float32)


# ------------------------------------------------------------- bass program
def _build_program(s_lo, s_hi, seq, qstream):
    import concourse.bacc as bacc
    import concourse.tile as tile
    import concourse.mybir as mybir
    from concourse import bass

    f32, i16, bf16 = mybir.dt.float32, mybir.dt.int16, mybir.dt.bfloat16
    AF = mybir.ActivationFunctionType
    OP = mybir.AluOpType
    S = s_lo + s_hi

    nc = bacc.Bacc("TRN2", target_bir_lowering=False, debug=False,
                   num_devices=NCORES, num_swdge_queues=4)
    hb_t = nc.dram_tensor("hb", [N, 128], bf16, kind="ExternalInput")
    hs_t = nc.dram_tensor("h_slice", [NPAD, F], f32, kind="ExternalInput")
    w_t = nc.dram_tensor("attn_w", [2 * F], f32, kind="ExternalInput")
    # cols 0:256 = gather idx (i16 bits in a bf16 container), 256:1280 = mask
    pk_t = nc.dram_tensor("packed", [S, 128, 256 + K * W], bf16,
                          kind="ExternalInput")
    SKP = -(-S * K // 128) * 128
    aw_t = nc.dram_tensor("aw", [128, SKP // 32], f32, kind="ExternalInput")
    # device-natural layout: [128 partitions, bank*colgroup*feature] so the
    # final store is 128 contiguous 12.5KB descriptors (the window-strided
    # layout cost 6272 x 256B descriptors); the host decodes the permutation
    out_t = nc.dram_tensor("out", [128, NBANK * 7 * F], f32,
                           kind="ExternalOutput")
    adr_t = nc.dram_tensor("adr", [NPAD], f32, kind="Internal")
    ta_t = nc.dram_tensor("ta", [NWIN + 1, F], f32, kind="Internal")
    a2_t = nc.dram_tensor("a2", [SKP, W], bf16, kind="Internal")

    def bc_ap(tensor, offset, ap):
        return bass.AP(tensor=tensor, offset=offset, ap=ap)

    with tile.TileContext(nc) as tc:
        with tc.tile_pool(name="const", bufs=1) as const, \
             tc.tile_pool(name="pre", bufs=1) as pre, \
             tc.tile_pool(name="ps", bufs=1, space="PSUM") as ps:

            # ---------------- constants
            w1f = const.tile([128, F], f32)
            nc.gpsimd.dma_start(out=w1f[:], in_=bc_ap(w_t, 0, [[0, 128], [1, F]]))
            w2t = const.tile([128, F], f32)
            nc.gpsimd.dma_start(out=w2t[:], in_=bc_ap(w_t, F, [[0, 128], [1, F]]))
            w1b = const.tile([128, F], bf16)
            nc.vector.tensor_copy(w1b[:], w1f[:])
            # physically replicated w1 over the chunk dim: keeps the zw
            # multiply's inputs step-1 contiguous so the DVE picks 2x mode
            # (a stride-0 broadcast AP drops it to 1x)
            w1r = const.tile([128, K, F], bf16)
            nc.vector.tensor_copy(
                w1r[:], w1b[:, None, :].to_broadcast([128, K, F]))

            # ---------------- preamble: a_dst table -> per-chunk A rows (bf16)
            with tc.tile_pool(name="pre2", bufs=1) as pre2:
                hs = pre2.tile([128, NPAD // 128, F], f32)
                nc.sync.dma_start(
                    out=hs[:], in_=hs_t[:].rearrange("(p t) f -> p t f", p=128))
                nc.vector.tensor_tensor(
                    out=hs[:], in0=hs[:],
                    in1=w2t[:, None, :].to_broadcast([128, NPAD // 128, F]),
                    op=OP.mult)
                a_sb = pre2.tile([128, NPAD // 128], f32)
                nc.vector.tensor_reduce(out=a_sb[:], in_=hs[:],
                                        axis=mybir.AxisListType.X, op=OP.add)
                nc.sync.dma_start(
                    out=adr_t[:].rearrange("(p t) -> p t", p=128), in_=a_sb[:])
                a_row = pre2.tile([1, NPAD], f32)
                nc.sync.dma_start(out=a_row[:],
                                  in_=bc_ap(adr_t, 0, [[0, 1], [1, NPAD]]))
                # ta_t row w = a_dst[32w .. 32w+32] (cols 32:64 unused);
                # row NWIN = zeros (dump chunks)
                nc.sync.dma_start(
                    out=ta_t[0:NWIN, 0:W],
                    in_=a_row[0:1, :].rearrange("p (w j) -> p w j", j=W))
                zrow = pre2.tile([128, W], f32)
                nc.vector.memset(zrow[:], 0.0)
                nc.sync.dma_start(out=ta_t[NWIN : NWIN + 1, 0:W],
                                  in_=zrow[0:1, :])
                # cols W:2W are gathered (256B elems) but unused — keep them
                # initialized so CoreSim's finiteness checks pass
                nc.sync.dma_start(
                    out=ta_t[:, W : 2 * W],
                    in_=zrow[0:1, None, :].to_broadcast([1, NWIN + 1, W]))
                awi = pre2.tile([128, SKP // 32], f32)
                nc.sync.dma_start(out=awi[:], in_=aw_t[:])
                At = pre2.tile([128, SKP // 128, F], f32)
                nc.gpsimd.dma_gather(
                    out_ap=At[:], in_ap=ta_t[:],
                    idxs_ap=awi[:].bitcast(i16), num_idxs=SKP,
                    num_idxs_reg=SKP, elem_size=F, queue_num=0)
                A2 = pre2.tile([128, SKP // 128, W], bf16)
                nc.vector.tensor_copy(A2[:], At[:, :, 0:W])
                nc.sync.dma_start(
                    out=a2_t[:].rearrange("(c p) w -> p c w", p=128),
                    in_=A2[:])

            # ---------------- resident PSUM window accumulators
            # window w -> bank w%7, slot w//7: partition group (w//7)%4,
            # col group (w//7)//4. bank 7 = dump target for pad chunks.
            banks = [ps.tile([128, 512], f32, name=f"bank{b}", tag=f"bank{b}")
                     for b in range(8)]

            def bank_region(w):
                if w == DUMP:
                    return banks[7][0:32, 0:65], (0, 0)
                slot = w // NBANK
                p0, c0 = 32 * (slot % 4), 65 * (slot // 4)
                return banks[w % NBANK][p0 : p0 + 32, c0 : c0 + 65], (0, p0)

            # ---------------- super-steps (2-stage software pipeline)
            # stage A(s): gathers + DVE score prep + scalar Lrelu/Exp
            # stage B(s): Sp = P*mask, scatter matmuls — emitted one
            # iteration later so the scalar round trip never stalls DVE.
            from contextlib import ExitStack
            lctx = ExitStack()
            ldi = lctx.enter_context(tc.tile_pool(name="ldi", bufs=5))
            zp = lctx.enter_context(tc.tile_pool(name="zp", bufs=5))
            b3 = lctx.enter_context(tc.tile_pool(name="b3", bufs=4))
            med = lctx.enter_context(tc.tile_pool(name="med", bufs=4))

            stash = {}
            for it in range(S + 1):
                if it < S:
                    s = it
                    ld = ldi.tile([128, 256 + K * W], bf16, tag="ld")
                    nc.sync.dma_start(out=ld[:], in_=pk_t[s])
                    ig = ld[:, 0:256].bitcast(i16)

                    Z = zp.tile([128, K, 128], bf16, tag="Z")
                    for q in range(4):
                        tab = (hb_t[0:HALF, :] if qstream[s][q] == 0
                               else hb_t[HALF:N, :])
                        nc.gpsimd.dma_gather(
                            out_ap=Z[:, 8 * q : 8 * q + 8, :],
                            in_ap=tab,
                            idxs_ap=ig[:, 64 * q : 64 * q + 64],
                            num_idxs=1024, num_idxs_reg=1024, elem_size=128,
                            queue_num=q)

                    Mt = ld[:, 256:].rearrange("p (c w) -> p c w", w=W)
                    # A[c, w] = a_dst[32*w_c + w], partition-replicated
                    A = med.tile([128, K, W], bf16, tag="A")
                    nc.sync.dma_start(
                        out=A[:],
                        in_=bc_ap(a2_t, s * K * W,
                                  [[0, 128], [W, K], [1, W]]))

                    # e = rowsum(Z * w1) — DVE reduce accumulates fp32
                    # internally; bf16 output rounding is within tolerance
                    zw = med.tile([128, K, F], bf16, tag="zw")
                    nc.vector.tensor_tensor(
                        out=zw[:], in0=Z[:, :, 0:F], in1=w1r[:],
                        op=OP.mult)
                    sCb = med.tile([128, K], bf16, tag="sCb")
                    with nc.allow_low_precision("bf16 scores within 2e-2 gate"):
                        nc.vector.tensor_reduce(out=sCb[:], in_=zw[:],
                                                axis=mybir.AxisListType.X,
                                                op=OP.add)

                    # E = e + a_dst; leaky = max(E, 0.01E) (ACT Copy shares
                    # Exp's table set — Lrelu does not and thrashes loads)
                    Emat = b3.tile([128, K, W], bf16, tag="Emat")
                    nc.vector.tensor_tensor(
                        out=Emat[:],
                        in0=sCb[:, :, None].to_broadcast([128, K, W]),
                        in1=A[:], op=OP.add)
                    El = b3.tile([128, K, W], bf16, tag="El")
                    nc.scalar.activation(out=El[:], in_=Emat[:], func=AF.Copy,
                                         scale=NEG_SLOPE)
                    nc.vector.tensor_tensor(out=El[:], in0=El[:], in1=Emat[:],
                                            op=OP.max)
                    Pm = b3.tile([128, K, W], bf16, tag="Pm")
                    nc.scalar.activation(out=Pm[:], in_=El[:], func=AF.Exp)
                    stash[s] = (Z, Pm, Mt)

                if it >= 1:
                    s = it - 1
                    Z, Pm, Mtv = stash.pop(s)
                    Sp = b3.tile([128, K, W], bf16, tag="Sp")
                    nc.vector.tensor_tensor(out=Sp[:], in0=Pm[:], in1=Mtv,
                                            op=OP.mult)
                    for c in range(K):
                        w, _i, _st, first, last = seq[s * K + c]
                        reg, tpos = bank_region(w)
                        nc.tensor.matmul(out=reg, lhsT=Sp[:, c, :],
                                         rhs=Z[:, c, 0:F + 1],
                                         start=first, stop=last,
                                         tile_position=tpos)

            # ---------------- epilogue: drain banks, divide by denominator
            acc = pre.tile([128, NBANK, 28 // 4 * 65], f32)
            for b in range(NBANK):
                nc.scalar.copy(out=acc[:, b, :], in_=banks[b][:, 0 : 455])
            accv = acc[:].rearrange("p b (k x) -> p b k x", x=65)
            rmax = pre.tile([128, NBANK, 7], f32)
            nc.vector.tensor_scalar_max(rmax[:], accv[:, :, :, F], 1e-30)
            rcp = pre.tile([128, NBANK, 7], f32)
            nc.vector.reciprocal(rcp[:], rmax[:])
            cmp = pre.tile([128, NBANK, 7, F], f32)
            nc.vector.tensor_tensor(
                out=cmp[:], in0=accv[:, :, :, 0:F],
                in1=rcp[:, :, :, None].to_broadcast([128, NBANK, 7, F]),
                op=OP.mult)
            nc.sync.dma_start(out=out_t[:], in_=cmp[:])
            lctx.close()
    nc.compile()
    return nc


_prog_cache = {}
_last_in_maps = None
_last_res = None


def kernel(h, attn_w, src, dst):
    from concourse.bass_utils import run_bass_kernel_spmd

    h = np.ascontiguousarray(h, dtype=np.float32)
    attn_w = np.ascontiguousarray(attn_w, dtype=np.float32)
    src = np.asarray(src, dtype=np.int32)
    dst = np.asarray(dst, dtype=np.int32)

    cores, nch = _prep(src, dst)
    seq, qstream, s_lo, s_hi = _schedule(nch)

    key = (s_lo, s_hi, tuple(seq), tuple(map(tuple, qstream)))
    if key not in _prog_cache:
        _prog_cache[key] = _build_program(s_lo, s_hi, seq, qstream)
    nc = _prog_cache[key]

    # bf16 gather table: row n = [h[n] | 1.0 | zeros]; col 64 is the
    # softmax-denominator ones column
    hb = np.zeros((N, 128), BF16)
    hb[:, :F] = h
    hb[:, F] = 1.0

    in_maps = []
    for d in range(NCORES):
        n0 = d * NLOC
        packed, aw = _build_arrays(cores[d], seq, qstream, s_lo, s_hi)
        h_slice = np.zeros((NPAD, F), np.float32)
        h_slice[:NLOC] = h[n0 : n0 + NLOC]
        in_maps.append({
            "hb": hb,
            "h_slice": h_slice,
            "attn_w": attn_w,
            "packed": packed,
            "aw": aw,
        })

    global _last_in_maps, _last_res
    _last_in_maps = in_maps
    res = run_bass_kernel_spmd(nc, in_maps, list(range(NCORES)))
    _last_res = res
    # device layout: out[g*32+r, b, k, f] holds window w = 7*(4k+g)+b row r
    g, r, b, k = np.meshgrid(np.arange(4), np.arange(32), np.arange(NBANK),
                             np.arange(7), indexing="ij")
    node = (32 * (NBANK * (4 * k + g) + b) + r).ravel()
    order = np.argsort(node)
    out = np.concatenate(
        [res.results[d]["out"].reshape(NPAD, F)[order[:NLOC]]
         for d in range(NCORES)])
    return out.astype(np.float32)


if __name__ == "__main__":
    import reference

    inputs = reference.setup_inputs()
    inputs = {k: np.asarray(v) for k, v in inputs.items()}
    got = kernel(**inputs)
    exp = np.asarray(reference.reference(**inputs))
    denom = np.abs(exp).max()
    rel = np.abs(got - exp).max() / denom
    print("Relative error:", rel)
